# revision 25
# baseline (speedup 1.0000x reference)
"""Trainium2 Bass kernel for nn_CausalAttention (which is actually full,
non-causal single-head attention: the reference's mask is all-False).

  q = x @ w_q.T ; k = x @ w_k.T ; v = x @ w_v.T        (per batch)
  out = softmax(q @ k.T / sqrt(512)) @ v

Shapes: x [4, 4096, 512], w_* [512, 512] fp32.

Sharding: 8 cores = 4 batches x 2 query-halves. Each core projects the
full K/V for its batch plus its 2048-query half of Q, then runs attention
for its queries against all 4096 keys.

Device layout is fully "transposed space" so no on-device transposes are
needed anywhere:
  - host supplies x^T [512, 4096] (d_in on partitions, bf16)
  - M = Wk^T Wq precompute folds both score projections:
    scores^T[s, t] = sum_i x^T[i, s] y[i, t],  y = M x^T   (bf16)
  - exp on ScalarE with fused 1/sqrt(512) scale -> e (bf16)
  - AV runs in fp8 DoubleRow (2x PE throughput) on the CENTERED weights
    g = e - 1: out[t, o] = Vsum[o] + sum_s g8[s, t] v8[s, o], where
    Vsum = Wv Xsum = exact sum_s v[s] (Xsum = sum_s x[s] via ScalarE
    accum reduction). Attention here is near-uniform (|scaled scores| <
    ~1.7, e ~= 1), so quantizing g (|g| ~ 0.35) instead of e (~1.05)
    cuts the fp8 noise ~3x; v8 noise also only enters multiplied by g.
    Measured: rel err 0.0126 vs 0.0045 all-bf16, gate 2e-2 (numpy sim
    matches HW bit-exactly).
  - AV keeps QUERIES on the psum partitions (lhsT=g8 stationary, rhs=v8
    moving), so softmax normalization is a per-partition tensor_scalar
    multiply: no partition_broadcast, transposed colsum via 4 tiny
    matmuls, and the Vsum offset folds in as rank-1 matmuls inside the
    psum accumulation group.
  - out is bf16 [2048, 512] row-major (host upcasts to fp32).

For core half=1 the host rotates x^T columns by 2048 so the program's
fixed "queries = columns 0..2047" holds; attention is invariant to key
order, so k/v built from the rotated x are equivalent.
"""

import math
import sys

for _p in ("/opt/trn_rl_repo",):
    if _p not in sys.path:
        sys.path.insert(0, _p)

import ml_dtypes
import numpy as np

import concourse.bass as bass
import concourse.tile as tile
from concourse import bacc, bass_isa, mybir
from concourse.bass_utils import run_bass_kernel_spmd

BF16 = ml_dtypes.bfloat16

B = 4            # batch
N = 4096         # sequence length
D = 512          # d_in = d_out
P = 128          # partitions
DC = D // P      # 4 chunks of the 512-dim on partitions
HALF = N // 2    # 2048 queries per core
TQ = 512         # query-tile width (matmul free dim)
NQT = HALF // TQ  # 4 query tiles per core
NST = N // P     # 32 key chunks of 128
SCALE = 1.0 / math.sqrt(float(D))
NCORES = 8

_f32 = mybir.dt.float32
_bf16 = mybir.dt.bfloat16
_f8e4 = mybir.dt.float8e4
_DR = mybir.MatmulPerfMode.DoubleRow


def _build_kernel():
    nc = bacc.Bacc(
        "TRN2", target_bir_lowering=False, debug=False, num_devices=NCORES
    )

    # wq/wk arrive in natural [d_out, d_in] layout (for the M = Wk^T Wq
    # precompute, which contracts over d_out); wv arrives transposed.
    xt = nc.dram_tensor("xt", [D, N], _bf16, kind="ExternalInput")
    # host-precast e4m3 copy of x^T dims 256..511 for the hybrid fp8 scores
    x8 = nc.dram_tensor("x8", [D // 2, N], _f8e4, kind="ExternalInput")
    wq = nc.dram_tensor("wq", [D, D], _bf16, kind="ExternalInput")
    wk = nc.dram_tensor("wk", [D, D], _bf16, kind="ExternalInput")
    wv = nc.dram_tensor("wv", [D, D], _bf16, kind="ExternalInput")
    # out in natural [query, d_out] layout (queries land on psum partitions
    # in the AV matmul), bf16: host upcasts to fp32.
    out = nc.dram_tensor("out", [HALF, D], _bf16, kind="ExternalOutput")

    # leading index l = c*128 + p  ->  partition p, free chunk c (consistent
    # everywhere a 512-dim sits on partitions)
    xt_r = xt[:, :].rearrange("(c p) n -> p c n", p=P)
    x8_r = x8[:, :].rearrange("(c p) n -> p c n", p=P)
    wq_r = wq[:, :].rearrange("(c p) o -> p c o", p=P)
    wk_r = wk[:, :].rearrange("(c p) o -> p c o", p=P)
    wv_r = wv[:, :].rearrange("(c p) o -> p c o", p=P)
    out_ap = out[:, :]

    with tile.TileContext(nc) as tc:
        with (
            tc.tile_pool(name="singles", bufs=1) as singles,
            tc.tile_pool(name="epool", bufs=8) as epool,
            tc.tile_pool(name="gpool", bufs=3) as gpool,
            tc.tile_pool(name="spool", bufs=2) as spool,
            tc.tile_pool(name="rpool", bufs=2) as rpool,
            tc.tile_pool(name="opool", bufs=4) as opool,
            tc.tile_pool(name="psA", bufs=4, space="PSUM") as psA,
            tc.tile_pool(name="psS", bufs=3, space="PSUM") as psS,
            tc.tile_pool(name="psC", bufs=1, space="PSUM") as psC,
        ):
            # ---- persistent SBUF tensors -------------------------------
            wq_sb = singles.tile([P, DC, D], _bf16, name="wq_sb")
            wk_sb = singles.tile([P, DC, D], _bf16, name="wk_sb")
            wv_sb = singles.tile([P, DC, D], _bf16, name="wv_sb")
            # First MT matmul needs only wk chunk 0 + wq column-chunk 0, so
            # land those two first.
            nc.sync.dma_start(wk_sb[:, 0], wk_r[:, 0])
            nc.sync.dma_start(wq_sb[:, :, 0:P], wq_r[:, :, 0:P])
            for oc in range(1, DC):
                nc.sync.dma_start(wk_sb[:, oc], wk_r[:, oc])
            for jc in range(1, DC):
                nc.sync.dma_start(
                    wq_sb[:, :, jc * P:(jc + 1) * P],
                    wq_r[:, :, jc * P:(jc + 1) * P],
                )

            # Load order matters: MT needs wq+wk (1MB), then the query half
            # in fine (d_in-chunk x 512-col) pieces for the y projection,
            # then wv and the key half for v. This gets the first matmul
            # issued after ~1MB of DMA.
            xt_sb = singles.tile([P, DC, N], _bf16, name="xt_sb")
            for tt in range(NQT):
                for c in range(DC):
                    sl = slice(tt * TQ, (tt + 1) * TQ)
                    nc.sync.dma_start(xt_sb[:, c, sl], xt_r[:, c, sl])
            nc.sync.dma_start(wv_sb[:], wv_r)
            for c in range(DC):
                nc.sync.dma_start(xt_sb[:, c, HALF:], xt_r[:, c, HALF:])
            xt8_sb = singles.tile([P, 2, N], _f8e4, name="xt8_sb")
            nc.sync.dma_start(xt8_sb[:], x8_r)

            # HAM warmup: the PE clock sits gated at 1.2GHz until ~8us of
            # sustained matmul activity (measured). The PE is otherwise idle
            # while the first weight DMAs land, so burn that window on
            # dependency-free dummy matmuls over memset data — emitted FIRST
            # so nothing delays them; sized to finish just before the weights
            # arrive so they never push the real matmuls out.
            warm_sb = singles.tile([P, TQ], _bf16, name="warm_sb")
            nc.vector.memset(warm_sb[:], 0.0)
            for wi in range(9):
                wps = psS.tile([P, TQ], _f32, tag="sc", name=f"warm_{wi}")
                nc.tensor.matmul(
                    wps[:], lhsT=warm_sb[:, :P], rhs=warm_sb[:],
                    start=True, stop=True,
                )

            ones_sb = singles.tile([P, 1], _f32, name="ones_sb")
            nc.gpsimd.memset(ones_sb[:], 1.0)
            ones_row = singles.tile([1, TQ], _bf16, name="ones_row")
            nc.gpsimd.memset(ones_row[:], 1.0)

            mt_sb = singles.tile([P, DC, D], _bf16, name="mt_sb")
            v8_sb = singles.tile([P, NST, D], _f8e4, name="v8_sb")
            y_sb = singles.tile([P, DC // 2, HALF], _bf16, name="y_sb")
            y8_sb = singles.tile([P, 2, HALF], _f8e4, name="y8_sb")
            xsum_f = singles.tile([P, DC], _f32, name="xsum_f")
            xsum_b = singles.tile([P, DC], _bf16, name="xsum_b")
            xscratch = singles.tile([P, N], _bf16, name="xscratch")
            vsum_row = singles.tile([1, D], _bf16, name="vsum_row")

            # ---- phase A: projections ----------------------------------
            # MT[j, i] = sum_o wq[o, j] * wk[o, i]  (= (Wk^T Wq)^T).
            # Folding the two score-side projections into one 512x512
            # precompute: scores^T = x^T^T (Wk^T Wq) x^T = x @ (M x^T).
            for jc in range(DC):
                ps = psA.tile([P, D], _f32, tag="ps")
                for oc in range(DC):
                    nc.tensor.matmul(
                        ps[:],
                        lhsT=wq_sb[:, oc, jc * P:(jc + 1) * P],
                        rhs=wk_sb[:, oc, :],
                        start=(oc == 0),
                        stop=(oc == DC - 1),
                    )
                nc.vector.tensor_copy(mt_sb[:, jc, :], ps[:])
            # y[i, t] = sum_j M[i, j] x^T[j, t] for our 2048 queries
            # (= columns 0..2047 of xt). tt outer: the first psum groups all
            # consume the first 512-column slice, the first DMA to land.
            for tt in range(NQT):
                for ic in range(DC):
                    ps = psA.tile([P, TQ], _f32, tag="ps")
                    for jc in range(DC):
                        nc.tensor.matmul(
                            ps[:],
                            lhsT=mt_sb[:, jc, ic * P:(ic + 1) * P],
                            rhs=xt_sb[:, jc, tt * TQ:(tt + 1) * TQ],
                            start=(jc == 0),
                            stop=(jc == DC - 1),
                        )
                    # Alternate PSUM->SBUF casts between VectorE and the
                    # (idle in phase A) ScalarE so neither cast chain gates
                    # psum slot recycling. Dims 0..255 go to bf16, dims
                    # 256..511 straight to e4m3 (single rounding from the
                    # fp32 psum) for the hybrid fp8 score matmul.
                    if ic < DC // 2:
                        dst = y_sb[:, ic, tt * TQ:(tt + 1) * TQ]
                    else:
                        dst = y8_sb[:, ic - DC // 2, tt * TQ:(tt + 1) * TQ]
                    if (tt * DC + ic) % 2 == 0:
                        nc.vector.tensor_copy(dst, ps[:])
                    else:
                        nc.scalar.copy(dst, ps[:])
            # v[s, o] natural layout (s on partitions per 128-chunk), cast
            # straight to fp8 for the DoubleRow AV.
            for st in range(NST):
                ps = psA.tile([P, D], _f32, tag="ps")
                for ic in range(DC):
                    nc.tensor.matmul(
                        ps[:],
                        lhsT=xt_sb[:, ic, st * P:(st + 1) * P],
                        rhs=wv_sb[:, ic, :],
                        start=(ic == 0),
                        stop=(ic == DC - 1),
                    )
                if st % 2 == 0:
                    nc.vector.tensor_copy(v8_sb[:, st, :], ps[:])
                else:
                    nc.scalar.copy(v8_sb[:, st, :], ps[:])

            # Xsum[i] = sum_s x[s, i] via ScalarE accumulate (free-dim
            # reduction), then Vsum = Wv Xsum as a [1, 512] psum row -
            # exactly sum_s v[s, :] in fp32/bf16 precision.
            for c in range(DC):
                nc.scalar.activation(
                    xscratch[:], xt_sb[:, c, :],
                    mybir.ActivationFunctionType.Copy,
                    accum_out=xsum_f[:, c:c + 1],
                )
            nc.vector.tensor_copy(xsum_b[:], xsum_f[:])
            vs_ps = psC.tile([1, D], _f32, tag="cs", name="vs_ps")
            for ic in range(DC):
                nc.tensor.matmul(
                    vs_ps[:],
                    lhsT=xsum_b[:, ic:ic + 1],
                    rhs=wv_sb[:, ic, :],
                    start=(ic == 0),
                    stop=(ic == DC - 1),
                )
            nc.vector.tensor_copy(vsum_row[:], vs_ps[:])

            # ---- phase B: attention ------------------------------------
            # AV is oriented with QUERIES on the output partitions:
            # out[t, o] = sum_s g8[s, t] v8[s, o]  (lhsT=g8 stationary,
            # rhs=v8 moving) so the softmax normalization is a per-partition
            # tensor_scalar multiply - no partition_broadcast and contiguous
            # row-major output DMAs.
            for qt in range(NQT):
                q_sl = slice(qt * TQ, (qt + 1) * TQ)
                out_ps = [
                    psA.tile([P, D], _f32, tag="ps", name=f"out_ps_{qt}_{ts}")
                    for ts in range(TQ // P)
                ]
                esum = spool.tile([P, TQ], _f32, tag="esum")

                # Software-pipelined, with matmuls batched by PE mode in
                # FOUR-chunk groups - [bf16 x8 | fp8-DR score x4] followed
                # later by the fp8-DR AV blocks - because a bf16<->fp8-DR
                # mode switch breaks the PE weight-load overlap (~+150ns
                # on the next matmul; fine-grained interleaving measured
                # 15us slower). The 4th score psum bank is borrowed from
                # the otherwise-idle psC pool.
                def emit_scores_quad(qr):
                    # scores^T[s, t] = sum_i x^T[i, s] y[i, t]; hybrid:
                    # dims 0..255 in two bf16 passes, dims 256..511 in one
                    # fp8 DoubleRow pass (error stays under the gate since
                    # only half the contraction is quantized).
                    sts = [4 * qr + j for j in range(4) if 4 * qr + j < NST]
                    scs = []
                    for k, st in enumerate(sts):
                        pool = psC if k == 3 else psS
                        sc = pool.tile(
                            [P, TQ], _f32, tag="cs" if k == 3 else "sc",
                            name=f"sc_{qt}_{st}",
                        )
                        scs.append(sc)
                        for dc in range(DC // 2):
                            nc.tensor.matmul(
                                sc[:],
                                lhsT=xt_sb[:, dc, st * P:(st + 1) * P],
                                rhs=y_sb[:, dc, q_sl],
                                start=(dc == 0),
                                stop=False,
                            )
                    es_quad = []
                    for k, st in enumerate(sts):
                        nc.tensor.matmul(
                            scs[k][:],
                            lhsT=xt8_sb[:, :, st * P:(st + 1) * P],
                            rhs=y8_sb[:, :, q_sl],
                            start=False,
                            stop=True,
                            perf_mode=_DR,
                            skip_group_check=True,
                        )
                        e = epool.tile([P, TQ], _bf16, tag="e")
                        nc.scalar.activation(
                            e[:], scs[k][:], mybir.ActivationFunctionType.Exp,
                            scale=SCALE,
                        )
                        es_quad.append(e)
                    return es_quad

                g8_tiles = {}
                cs_box = []

                def emit_av(st, e):
                    if st == 0:
                        nc.vector.tensor_copy(esum[:], e[:])
                    else:
                        nc.vector.tensor_add(esum[:], esum[:], e[:])
                    pair = st // 2
                    if st % 2 == 0:
                        g8_tiles[pair] = gpool.tile(
                            [P, 2, TQ], _f8e4, tag="g8",
                            name=f"g8_{qt}_{pair}",
                        )
                    g8 = g8_tiles[pair]
                    # center: g = e - 1 quantized to e4m3 (fp8 noise scales
                    # with |g| ~ 0.35 instead of |e| ~ 1.05)
                    nc.vector.tensor_scalar_sub(g8[:, st % 2, :], e[:], 1.0)
                    if st % 2 == 0:
                        return
                    if st == NST - 1:
                        # rank-1 Vsum psum-add + transposed colsum (queries
                        # on partitions), tucked before the last AV group so
                        # the recip chain overlaps the final AV matmuls.
                        # cs is allocated HERE (after the last borrowed
                        # score bank) to keep the psC pool rotation in
                        # allocation = use order.
                        cs = psC.tile(
                            [P, TQ // P], _f32, tag="cs", name=f"cs_{qt}"
                        )
                        cs_box.append(cs)
                        for ts in range(TQ // P):
                            nc.tensor.matmul(
                                out_ps[ts][:],
                                lhsT=ones_row[:, 0:P],
                                rhs=vsum_row[:],
                                start=False, stop=False,
                                skip_group_check=True,
                            )
                        for ts in range(TQ // P):
                            nc.tensor.matmul(
                                cs[:, ts:ts + 1],
                                lhsT=esum[:, ts * P:(ts + 1) * P],
                                rhs=ones_sb[:],
                                start=True, stop=True,
                            )
                    for ts in range(TQ // P):
                        nc.tensor.matmul(
                            out_ps[ts][:],
                            lhsT=g8[:, :, ts * P:(ts + 1) * P],
                            rhs=v8_sb[:, st - 1:st + 1, :],
                            start=(st == 1),
                            stop=(st == NST - 1),
                            perf_mode=_DR,
                            skip_group_check=True,
                        )
                    del g8_tiles[pair]

                es = [*emit_scores_quad(0)]
                for st in range(NST):
                    if st % 4 == 0 and st // 4 + 1 < NST // 4:
                        es.extend(emit_scores_quad(st // 4 + 1))
                    emit_av(st, es[st])
                recip = rpool.tile([P, TQ // P], _f32, tag="recip")
                nc.vector.reciprocal_approx_fast(recip[:], cs_box[0][:])
                # Normalize + cast to bf16 split across VectorE/ScalarE, DMA
                # triggers split across SP/SWDGE queues, so the tail chain
                # after the last AV matmul is ~2 ops deep per engine.
                for ts in range(TQ // P):
                    ot = opool.tile([P, D], _bf16, tag="ot")
                    if ts % 2 == 0:
                        nc.vector.tensor_scalar_mul(
                            ot[:], out_ps[ts][:], recip[:, ts:ts + 1]
                        )
                    else:
                        # ScalarE (idle at the tail): out = in * scale[p]
                        nc.scalar.mul(ot[:], out_ps[ts][:], recip[:, ts:ts + 1])
                    deng = nc.sync if ts % 2 == 0 else nc.gpsimd
                    deng.dma_start(
                        out_ap[qt * TQ + ts * P:qt * TQ + (ts + 1) * P, :],
                        ot[:],
                    )

    nc.compile()
    return nc


_cached_nc = None
last_results = None  # BassKernelResults of the most recent run (for test.py)


def kernel(x, w_q, w_k, w_v):
    global _cached_nc, last_results
    if _cached_nc is None:
        _cached_nc = _build_kernel()
    nc = _cached_nc

    wq_n = np.ascontiguousarray(np.asarray(w_q, np.float32)).astype(BF16)
    wk_n = np.ascontiguousarray(np.asarray(w_k, np.float32)).astype(BF16)
    wv_t = np.ascontiguousarray(np.asarray(w_v, np.float32).T).astype(BF16)

    x = np.asarray(x, np.float32)
    in_maps = []
    for core in range(NCORES):
        b, h = core // 2, core % 2
        xT = np.ascontiguousarray(x[b].T).astype(BF16)  # [512, 4096]
        if h == 1:
            xT = np.ascontiguousarray(
                np.concatenate([xT[:, HALF:], xT[:, :HALF]], axis=1)
            )
        x8_c = np.ascontiguousarray(xT[D // 2:]).astype(
            ml_dtypes.float8_e4m3
        )
        in_maps.append(
            {"xt": xT, "x8": x8_c, "wq": wq_n, "wk": wk_n, "wv": wv_t}
        )

    res = run_bass_kernel_spmd(nc, in_maps, core_ids=list(range(NCORES)))
    last_results = res

    out = np.empty((B, N, D), np.float32)
    for core in range(NCORES):
        b, h = core // 2, core % 2
        out[b, h * HALF:(h + 1) * HALF, :] = res.results[core]["out"]
    return out
